# revision 15
# baseline (speedup 1.0000x reference)
"""Bundle-adjustment projection kernel for Trainium2 (8 NeuronCores).

Computes, for V=128 camera views and N=250000 3D points:
    R = euler_to_R(eulers);  Xc = R @ p + t
    u = -f*X/Z + CX;  v = f*Y/Z + CY;  outputs (uv[V,N,2], Z[V,N])

Strategy:
  * Shard points N across the 8 cores (31250 each, padded to 31744 = 62*512).
    Every core holds all 128 views' parameters (tiny).
  * Fold focal/CX/CY into per-view linear maps on the host:
        u = Pu/Z, v = Pv/Z with
        Pu = (-f*R0 + CX*R2) . p + (-f*t0 + CX*t2),  Z = R2 . p + t2, etc.
  * On device, each [u, v, Z] plane for a 512-point chunk is ONE matmul:
    out[view, point] = lhsT[K=16, view].T @ rhs[K=16, point] where the
    K=16 contraction implements a bf16 hi/lo split (full ~fp32 precision
    at full PE speed): (Whi+Wlo) . (phi+plo) cross terms + bias via an
    all-ones row.
  * Pu and Pv land side by side in one 2-bank PSUM tile, so the division
    is ONE DVE pass: tensor_mul((128,1024), in1 = recip(Z) read twice via
    a stride-0 access pattern). ACT copies Z out of PSUM. Outputs stage in
    SBUF and leave as contiguous multi-MB DMAs; u/v are de-interleaved on
    the host.
"""

import sys

sys.path.insert(0, "/opt/trn_rl_repo")

import numpy as np
import ml_dtypes

import concourse.bass as bass
import concourse.bacc as bacc
import concourse.mybir as mybir
from concourse.tile import TileContext
from concourse.bass_utils import run_bass_kernel_spmd

AF = mybir.ActivationFunctionType

CX = 512.0
CY = 512.0
V = 128
NPTS = 250000
NCORES = 8
NPC = NPTS // NCORES  # 31250
CHUNK = 512
NCHUNK = -(-NPC // CHUNK)  # 62
NPAD = NCHUNK * CHUNK  # 31744
GROUP = 4  # chunks per staged output DMA

BF16 = ml_dtypes.bfloat16

_nc_cache = None


def _build_nc():
    nc = bacc.Bacc(None, target_bir_lowering=False)

    pts = nc.dram_tensor("pts", [16, NPAD], mybir.dt.bfloat16, kind="ExternalInput")
    wts = nc.dram_tensor("wts", [16, 3 * V], mybir.dt.bfloat16, kind="ExternalInput")
    # interleaved per-chunk [u(512) | v(512)] planes
    ouv = nc.dram_tensor("ouv", [V, 2 * NPAD], mybir.dt.float32, kind="ExternalOutput")
    oz = nc.dram_tensor("oz", [V, NPAD], mybir.dt.float32, kind="ExternalOutput")

    ngroups = -(-NCHUNK // GROUP)

    with TileContext(nc) as tc:
        with (
            tc.tile_pool(name="wpool", bufs=1) as wpool,
            tc.tile_pool(name="rpool", bufs=3) as rpool,
            tc.tile_pool(name="stpool", bufs=3) as stpool,
            tc.tile_pool(name="psuv", bufs=2, space="PSUM") as psuv_pool,
            tc.tile_pool(name="psz", bufs=2, space="PSUM") as psz_pool,
        ):
            w = wpool.tile([16, 3 * V], mybir.dt.bfloat16, tag="w")
            nc.scalar.dma_start(out=w, in_=wts[:, :])

            # all points resident in SBUF; 4 up-front DMAs, matmuls slice
            # directly (subtile deps let chunk c start once its quarter lands)
            pts_sb = wpool.tile([16, NPAD], mybir.dt.bfloat16, tag="pts_sb")
            QTR = NPAD // 4
            for q in range(4):
                nc.scalar.dma_start(
                    out=pts_sb[:, q * QTR : (q + 1) * QTR],
                    in_=pts[:, q * QTR : (q + 1) * QTR],
                )

            for g in range(ngroups):
                glen = min(GROUP, NCHUNK - g * GROUP)
                gcols = glen * CHUNK
                gn0 = g * GROUP * CHUNK
                uv_st = stpool.tile(
                    [V, GROUP * 2 * CHUNK], mybir.dt.float32, tag="uv_st"
                )
                z_st = stpool.tile([V, GROUP * CHUNK], mybir.dt.float32, tag="z_st")
                for ci in range(glen):
                    off = ci * CHUNK
                    sl = slice(off, off + CHUNK)
                    asl = slice(gn0 + off, gn0 + off + CHUNK)
                    ps_uv = psuv_pool.tile([V, 2 * CHUNK], mybir.dt.float32, tag="ps_uv")
                    ps_z = psz_pool.tile([V, CHUNK], mybir.dt.float32, tag="ps_z")
                    nc.tensor.matmul(
                        ps_z[:, :], w[:, 2 * V : 3 * V], pts_sb[:, asl], start=True, stop=True
                    )
                    nc.tensor.matmul(
                        ps_uv[:, 0:CHUNK], w[:, 0:V], pts_sb[:, asl], start=True, stop=True
                    )
                    nc.tensor.matmul(
                        ps_uv[:, CHUNK : 2 * CHUNK],
                        w[:, V : 2 * V],
                        pts_sb[:, asl],
                        start=True,
                        stop=True,
                    )
                    r = rpool.tile([V, CHUNK], mybir.dt.float32, tag="r")
                    nc.vector.reciprocal_approx_fast(out=r, in_=ps_z)
                    nc.scalar.copy(out=z_st[:, sl], in_=ps_z[:, :])
                    r_b = bass.AP(r.tensor, r.offset, [r.ap[0], [0, 2], [1, CHUNK]])
                    nc.vector.tensor_mul(
                        out=uv_st[:, 2 * off : 2 * off + 2 * CHUNK],
                        in0=ps_uv[:, :],
                        in1=r_b,
                    )
                nc.sync.dma_start(
                    out=ouv[:, 2 * gn0 : 2 * gn0 + 2 * gcols], in_=uv_st[:, : 2 * gcols]
                )
                nc.sync.dma_start(out=oz[:, gn0 : gn0 + gcols], in_=z_st[:, :gcols])

    nc.compile()
    return nc


def _split_bf16(x):
    """x (fp64/fp32) -> (hi, lo) bf16 arrays with hi+lo ~ x to ~2^-18 rel."""
    x = np.asarray(x, dtype=np.float64)
    hi = x.astype(BF16)
    lo = (x - hi.astype(np.float64)).astype(BF16)
    return hi, lo


def _host_weights(focal, eulers, trans):
    f = float(np.asarray(focal, dtype=np.float64))
    e = np.asarray(eulers, dtype=np.float64)
    t = np.asarray(trans, dtype=np.float64)
    a, b, c = e[:, 0], e[:, 1], e[:, 2]
    ca, sa = np.cos(a), np.sin(a)
    cb, sb = np.cos(b), np.sin(b)
    cc, sc = np.cos(c), np.sin(c)
    one = np.ones_like(a)
    zero = np.zeros_like(a)

    def m3(r00, r01, r02, r10, r11, r12, r20, r21, r22):
        return np.stack(
            [
                np.stack([r00, r01, r02], axis=-1),
                np.stack([r10, r11, r12], axis=-1),
                np.stack([r20, r21, r22], axis=-1),
            ],
            axis=-2,
        )

    Rx = m3(one, zero, zero, zero, ca, -sa, zero, sa, ca)
    Ry = m3(cb, zero, sb, zero, one, zero, -sb, zero, cb)
    Rz = m3(cc, -sc, zero, sc, cc, zero, zero, zero, one)
    R = Rx @ Ry @ Rz  # (V, 3, 3)

    A_u = -f * R[:, 0, :] + CX * R[:, 2, :]  # (V, 3)
    b_u = -f * t[:, 0] + CX * t[:, 2]  # (V,)
    A_v = f * R[:, 1, :] + CY * R[:, 2, :]
    b_v = f * t[:, 1] + CY * t[:, 2]
    A_z = R[:, 2, :]
    b_z = t[:, 2]

    def lhsT(A, b):
        W = np.concatenate([A.T, b[None, :]], axis=0)  # (4, V)
        hi, lo = _split_bf16(W)
        return np.concatenate([hi, hi, lo, lo], axis=0)  # (16, V)

    return np.concatenate(
        [lhsT(A_u, b_u), lhsT(A_v, b_v), lhsT(A_z, b_z)], axis=1
    )  # (16, 3V) bf16


def _host_points(points3d):
    """points3d (N,3) fp32 -> per-core rhs arrays (16, NPAD) bf16."""
    p = np.asarray(points3d, dtype=np.float32)
    rhss = []
    for k in range(NCORES):
        sl = p[k * NPC : (k + 1) * NPC]
        pad = np.zeros((NPAD, 3), dtype=np.float32)
        pad[:NPC] = sl
        hi, lo = _split_bf16(pad.T)  # (3, NPAD) each
        blk = np.empty((8, NPAD), dtype=BF16)
        blk[0:3] = hi
        blk[3] = np.ones((NPAD,), dtype=BF16)
        blk[4:7] = lo
        blk[7] = np.zeros((NPAD,), dtype=BF16)
        rhss.append(np.concatenate([blk, blk], axis=0))  # (16, NPAD)
    return rhss


def _run(focal, eulers, trans, points3d, trace=False):
    global _nc_cache
    if _nc_cache is None:
        _nc_cache = _build_nc()
    nc = _nc_cache

    wts = _host_weights(focal, eulers, trans)
    rhss = _host_points(points3d)
    in_maps = [{"pts": rhss[k], "wts": wts} for k in range(NCORES)]

    res = run_bass_kernel_spmd(nc, in_maps, core_ids=list(range(NCORES)), trace=trace)

    uv = np.empty((V, NPTS, 2), dtype=np.float32)
    Z = np.empty((V, NPTS), dtype=np.float32)
    for k in range(NCORES):
        r = res.results[k]
        s = slice(k * NPC, (k + 1) * NPC)
        ouv = r["ouv"].reshape(V, NCHUNK, 2, CHUNK)
        uv[:, s, 0] = ouv[:, :, 0, :].reshape(V, NPAD)[:, :NPC]
        uv[:, s, 1] = ouv[:, :, 1, :].reshape(V, NPAD)[:, :NPC]
        Z[:, s] = r["oz"][:, :NPC]
    return (uv, Z), res


def kernel(focal, eulers, trans, points3d):
    out, _ = _run(focal, eulers, trans, points3d, trace=False)
    return out


# revision 17
# speedup vs baseline: 1.1905x; 1.1905x over previous
"""Bundle-adjustment projection kernel for Trainium2 (8 NeuronCores).

Computes, for V=128 camera views and N=250000 3D points:
    R = euler_to_R(eulers);  Xc = R @ p + t
    u = -f*X/Z + CX;  v = f*Y/Z + CY;  outputs (uv[V,N,2], Z[V,N])

Strategy:
  * Shard points N across the 8 cores (31250 each, padded to 31744 = 62*512).
    Every core holds all 128 views' parameters (tiny).
  * Fold focal/CX/CY into per-view linear maps on the host:
        u = Pu/Z, v = Pv/Z with
        Pu = (-f*R0 + CX*R2) . p + (-f*t0 + CX*t2),  Z = R2 . p + t2, etc.
  * On device, each [u, v, Z] plane for a 512-point chunk is ONE matmul:
    out[view, point] = lhsT[K=16, view].T @ rhs[K=16, point] where the
    K=16 contraction implements a bf16 hi/lo split (full ~fp32 precision
    at full PE speed): (Whi+Wlo) . (phi+plo) cross terms + bias via an
    all-ones row.
  * Pu and Pv land side by side in one 2-bank PSUM tile, so the division
    is ONE DVE pass: tensor_mul((128,1024), in1 = recip(Z) read twice via
    a stride-0 access pattern). ACT copies Z out of PSUM. Outputs stage in
    SBUF and leave as contiguous multi-MB DMAs; u/v are de-interleaved on
    the host.
"""

import sys

sys.path.insert(0, "/opt/trn_rl_repo")

import numpy as np
import ml_dtypes

import concourse.bass as bass
import concourse.bacc as bacc
import concourse.mybir as mybir
from concourse.tile import TileContext
from concourse.bass_utils import run_bass_kernel_spmd

AF = mybir.ActivationFunctionType

CX = 512.0
CY = 512.0
V = 128
NPTS = 250000
NCORES = 8
NPC = NPTS // NCORES  # 31250
CHUNK = 512
NCHUNK = -(-NPC // CHUNK)  # 62
NPAD = NCHUNK * CHUNK  # 31744
GROUP = 4  # chunks per staged output DMA

BF16 = ml_dtypes.bfloat16

_nc_cache = None


def _build_nc():
    nc = bacc.Bacc(None, target_bir_lowering=False)

    pts = nc.dram_tensor("pts", [16, NPAD], mybir.dt.bfloat16, kind="ExternalInput")
    wts = nc.dram_tensor("wts", [16, 3 * V], mybir.dt.bfloat16, kind="ExternalInput")
    # interleaved per-chunk [u(512) | v(512)] planes
    ouv = nc.dram_tensor("ouv", [V, 2 * NPAD], mybir.dt.float32, kind="ExternalOutput")
    oz = nc.dram_tensor("oz", [V, NPAD], mybir.dt.float32, kind="ExternalOutput")

    ngroups = -(-NCHUNK // GROUP)

    with TileContext(nc) as tc:
        with (
            tc.tile_pool(name="wpool", bufs=1) as wpool,
            tc.tile_pool(name="rhspool", bufs=3) as rhspool,
            tc.tile_pool(name="rpool", bufs=3) as rpool,
            tc.tile_pool(name="stpool", bufs=3) as stpool,
            tc.tile_pool(name="psuv", bufs=2, space="PSUM") as psuv_pool,
            tc.tile_pool(name="psz", bufs=4, space="PSUM") as psz_pool,
        ):
            w = wpool.tile([16, 3 * V], mybir.dt.bfloat16, tag="w")
            nc.scalar.dma_start(out=w, in_=wts[:, :])

            rhs_tiles = {}

            def ensure_rhs(g):
                if g in rhs_tiles or g >= ngroups:
                    return
                glen = min(GROUP, NCHUNK - g * GROUP)
                gcols = glen * CHUNK
                gn0 = g * GROUP * CHUNK
                t = rhspool.tile([16, GROUP * CHUNK], mybir.dt.bfloat16, tag="rhs")
                nc.scalar.dma_start(out=t[:, :gcols], in_=pts[:, gn0 : gn0 + gcols])
                rhs_tiles[g] = t

            psz_tiles = {}

            def emit_z(c):
                # Z matmul runs one chunk ahead of the uv pipeline so the
                # in-order DVE never stalls on a fresh reciprocal input.
                if c >= NCHUNK or c in psz_tiles:
                    return
                g, ci = divmod(c, GROUP)
                ensure_rhs(g)
                t = psz_pool.tile([V, CHUNK], mybir.dt.float32, tag="ps_z")
                nc.tensor.matmul(
                    t[:, :],
                    w[:, 2 * V : 3 * V],
                    rhs_tiles[g][:, ci * CHUNK : (ci + 1) * CHUNK],
                    start=True,
                    stop=True,
                )
                psz_tiles[c] = t

            uv_st = None
            z_st = None
            emit_z(0)
            for c in range(NCHUNK):
                g, ci = divmod(c, GROUP)
                glen = min(GROUP, NCHUNK - g * GROUP)
                gcols = glen * CHUNK
                gn0 = g * GROUP * CHUNK
                if ci == 0:
                    uv_st = stpool.tile(
                        [V, GROUP * 2 * CHUNK], mybir.dt.float32, tag="uv_st"
                    )
                    z_st = stpool.tile([V, GROUP * CHUNK], mybir.dt.float32, tag="z_st")
                off = ci * CHUNK
                sl = slice(off, off + CHUNK)
                emit_z(c + 1)
                ps_uv = psuv_pool.tile([V, 2 * CHUNK], mybir.dt.float32, tag="ps_uv")
                nc.tensor.matmul(
                    ps_uv[:, 0:CHUNK],
                    w[:, 0:V],
                    rhs_tiles[g][:, sl],
                    start=True,
                    stop=True,
                )
                nc.tensor.matmul(
                    ps_uv[:, CHUNK : 2 * CHUNK],
                    w[:, V : 2 * V],
                    rhs_tiles[g][:, sl],
                    start=True,
                    stop=True,
                )
                ps_z = psz_tiles.pop(c)
                r = rpool.tile([V, CHUNK], mybir.dt.float32, tag="r")
                nc.vector.reciprocal_approx_fast(out=r, in_=ps_z)
                nc.scalar.copy(out=z_st[:, sl], in_=ps_z[:, :])
                r_b = bass.AP(r.tensor, r.offset, [r.ap[0], [0, 2], [1, CHUNK]])
                nc.vector.tensor_mul(
                    out=uv_st[:, 2 * off : 2 * off + 2 * CHUNK],
                    in0=ps_uv[:, :],
                    in1=r_b,
                )
                if ci == glen - 1:
                    nc.sync.dma_start(
                        out=ouv[:, 2 * gn0 : 2 * gn0 + 2 * gcols],
                        in_=uv_st[:, : 2 * gcols],
                    )
                    nc.sync.dma_start(out=oz[:, gn0 : gn0 + gcols], in_=z_st[:, :gcols])
                    rhs_tiles.pop(g, None)

    nc.compile()
    return nc


def _split_bf16(x):
    """x (fp64/fp32) -> (hi, lo) bf16 arrays with hi+lo ~ x to ~2^-18 rel."""
    x = np.asarray(x, dtype=np.float64)
    hi = x.astype(BF16)
    lo = (x - hi.astype(np.float64)).astype(BF16)
    return hi, lo


def _host_weights(focal, eulers, trans):
    f = float(np.asarray(focal, dtype=np.float64))
    e = np.asarray(eulers, dtype=np.float64)
    t = np.asarray(trans, dtype=np.float64)
    a, b, c = e[:, 0], e[:, 1], e[:, 2]
    ca, sa = np.cos(a), np.sin(a)
    cb, sb = np.cos(b), np.sin(b)
    cc, sc = np.cos(c), np.sin(c)
    one = np.ones_like(a)
    zero = np.zeros_like(a)

    def m3(r00, r01, r02, r10, r11, r12, r20, r21, r22):
        return np.stack(
            [
                np.stack([r00, r01, r02], axis=-1),
                np.stack([r10, r11, r12], axis=-1),
                np.stack([r20, r21, r22], axis=-1),
            ],
            axis=-2,
        )

    Rx = m3(one, zero, zero, zero, ca, -sa, zero, sa, ca)
    Ry = m3(cb, zero, sb, zero, one, zero, -sb, zero, cb)
    Rz = m3(cc, -sc, zero, sc, cc, zero, zero, zero, one)
    R = Rx @ Ry @ Rz  # (V, 3, 3)

    A_u = -f * R[:, 0, :] + CX * R[:, 2, :]  # (V, 3)
    b_u = -f * t[:, 0] + CX * t[:, 2]  # (V,)
    A_v = f * R[:, 1, :] + CY * R[:, 2, :]
    b_v = f * t[:, 1] + CY * t[:, 2]
    A_z = R[:, 2, :]
    b_z = t[:, 2]

    def lhsT(A, b):
        W = np.concatenate([A.T, b[None, :]], axis=0)  # (4, V)
        hi, lo = _split_bf16(W)
        return np.concatenate([hi, hi, lo, lo], axis=0)  # (16, V)

    return np.concatenate(
        [lhsT(A_u, b_u), lhsT(A_v, b_v), lhsT(A_z, b_z)], axis=1
    )  # (16, 3V) bf16


def _host_points(points3d):
    """points3d (N,3) fp32 -> per-core rhs arrays (16, NPAD) bf16."""
    p = np.asarray(points3d, dtype=np.float32)
    rhss = []
    for k in range(NCORES):
        sl = p[k * NPC : (k + 1) * NPC]
        pad = np.zeros((NPAD, 3), dtype=np.float32)
        pad[:NPC] = sl
        hi, lo = _split_bf16(pad.T)  # (3, NPAD) each
        blk = np.empty((8, NPAD), dtype=BF16)
        blk[0:3] = hi
        blk[3] = np.ones((NPAD,), dtype=BF16)
        blk[4:7] = lo
        blk[7] = np.zeros((NPAD,), dtype=BF16)
        rhss.append(np.concatenate([blk, blk], axis=0))  # (16, NPAD)
    return rhss


def _run(focal, eulers, trans, points3d, trace=False):
    global _nc_cache
    if _nc_cache is None:
        _nc_cache = _build_nc()
    nc = _nc_cache

    wts = _host_weights(focal, eulers, trans)
    rhss = _host_points(points3d)
    in_maps = [{"pts": rhss[k], "wts": wts} for k in range(NCORES)]

    res = run_bass_kernel_spmd(nc, in_maps, core_ids=list(range(NCORES)), trace=trace)

    uv = np.empty((V, NPTS, 2), dtype=np.float32)
    Z = np.empty((V, NPTS), dtype=np.float32)
    for k in range(NCORES):
        r = res.results[k]
        s = slice(k * NPC, (k + 1) * NPC)
        ouv = r["ouv"].reshape(V, NCHUNK, 2, CHUNK)
        uv[:, s, 0] = ouv[:, :, 0, :].reshape(V, NPAD)[:, :NPC]
        uv[:, s, 1] = ouv[:, :, 1, :].reshape(V, NPAD)[:, :NPC]
        Z[:, s] = r["oz"][:, :NPC]
    return (uv, Z), res


def kernel(focal, eulers, trans, points3d):
    out, _ = _run(focal, eulers, trans, points3d, trace=False)
    return out
